# revision 1
# baseline (speedup 1.0000x reference)
"""CRF log-likelihood kernel for Trainium2 (Bass/Tile), 8-core data parallel.

v2: segmented rank-1 scan.

The forward algorithm runs in the exp domain: alpha' = E_t * (W @ alpha),
with W = exp(trans)^T stationary and E_t per-step emission factors
(pre-normalized so values stay in bf16 range; dead timesteps park the
state at STOP via E = e_STOP and trans[STOP,STOP]=0).

Because each step is linear in alpha, the 1024-step serial chain is cut
into S=32 segments of 32 steps. A segment's transfer matrix is a product
of 32 dense positive matrices, which contracts to numerical rank-1
(sigma2/sigma1 < 1e-10), so seg s is represented by one forward probe
f_s = M_s p and one backward probe g_s^T = q^T M_s. The chains advance
in lockstep as two independent 16-segment halves ([128x512] bf16 matmul
+ [128,512] DVE multiply per direction per half per slot); half 0 only
needs chunks 0..3 of E, so its emission is interleaved with the chunk
4..7 staging and overlaps phase 1. Serial depth drops 1025 -> 32. The
telescoped product

  partition ~ ln(g_31.f_30) + sum_{k=1}^{30} [ln(g_k+1 . f_k) - ln(sum f_k)]

is computed with per-sequence columnwise dots. Validated: 4.6e-5 output
rel err vs fp64 (tolerance 2e-2).

Gold score: host precomputes int16 gather indices; gpsimd ap_gather
fetches x[b,t,tag] (from the staged x chunk, per chunk) and
trans[tag_t,tag_prev] (one 4128-index gather from a replicated bf16
table, stop transitions folded in); host-built masks select the valid
(diagonal, parity, x_mask) lanes; per-sequence totals reduce via Act
accum_out. The emission's baked-in ln(ik) normalizer cancels exactly
against the partition's, so neither x_len nor kappa appears on device.

x ships host-side as bf16, t-major [L, BC, 128], padded so that
exp(x_pad) directly yields the scan's E factors (mask, STOP parking
column, and normalizer all baked in by the host).
"""

import os

import numpy as np
import ml_dtypes

import concourse.bass as bass
import concourse.bacc as bacc
import concourse.mybir as mybir
import concourse.tile as tile
from concourse.bass_utils import run_bass_kernel_spmd

F32 = mybir.dt.float32
BF16 = mybir.dt.bfloat16
I16 = mybir.dt.int16
I32 = mybir.dt.int32
AX = mybir.AxisListType
OP = mybir.AluOpType
ACT = mybir.ActivationFunctionType

B_FULL, L_FULL, D = 256, 1024, 126
T = 128
START, STOP = 126, 127
N_CORES = 8
BC = B_FULL // N_CORES          # 32 sequences per core
NCH = L_FULL // 128             # 8 chunks
S = 32                          # segments
SEG = L_FULL // S               # 32 steps per segment
NB = S * BC                     # 1024 columns per scan bank

BF = ml_dtypes.bfloat16


def build_nc():
    nc = bacc.Bacc(None)

    xb_d = nc.dram_tensor("x_bf", [L_FULL, BC, T], BF16, kind="ExternalInput")
    tr_d = nc.dram_tensor("transitions", [T, T], F32, kind="ExternalInput")
    trep_d = nc.dram_tensor("trep", [T, T * T], BF16, kind="ExternalInput")
    gmask_d = nc.dram_tensor("gmask", [NCH, T, 1024], BF16, kind="ExternalInput")
    eidx_d = nc.dram_tensor("eidx", [NCH, T, 32], I16, kind="ExternalInput")
    pidx_d = nc.dram_tensor("pidx", [T, 258], I16, kind="ExternalInput")
    pmask_d = nc.dram_tensor("pmask", [T, 8256], BF16, kind="ExternalInput")
    out_d = nc.dram_tensor("out", [BC], F32, kind="ExternalOutput")

    with tile.TileContext(nc) as tc:
        with (
            tc.tile_pool(name="const", bufs=1) as cpool,
            tc.tile_pool(name="work", bufs=2) as wpool,
            tc.tile_pool(name="scan", bufs=2) as apool,
            tc.tile_pool(name="psT", bufs=2, space="PSUM") as psT,
            tc.tile_pool(name="psA", bufs=1, space="PSUM") as psA,
            tc.tile_pool(name="psB", bufs=1, space="PSUM") as psB,
            tc.tile_pool(name="psM", bufs=2, space="PSUM") as psM,
        ):
            # ---------------- constants ----------------
            trans_sb = cpool.tile([T, T], F32)
            nc.sync.dma_start(trans_sb[:], tr_d[:])
            transT_sb = cpool.tile([T, T], F32)
            nc.sync.dma_start(transT_sb[:], tr_d[:].rearrange("i j -> j i"))
            # (trep/pidx/pmask DMAs are issued late — after the x chunks —
            # so the serialized DMA resource feeds the scan's E buffer first)
            trep_sb = cpool.tile([T, T * T], BF16)
            pidx_sb = cpool.tile([T, 258], I16)
            pmask_sb = cpool.tile([T, 8256], BF16)

            # W_sb[j,i] = exp(trans[i,j])  (fwd stationary)
            W_sb = cpool.tile([T, T], BF16)
            nc.scalar.activation(W_sb[:], transT_sb[:], ACT.Exp)
            # W2_sb[j,i] = exp(trans[j,i]) (bwd stationary)
            W2_sb = cpool.tile([T, T], BF16)
            nc.scalar.activation(W2_sb[:], trans_sb[:], ACT.Exp)
            # r[j] = exp(trans[STOP, j]) = exp(transT[j, STOP])
            rcol = cpool.tile([T, 1], BF16)
            nc.scalar.activation(rcol[:], transT_sb[:, STOP : STOP + 1], ACT.Exp)

            iota_ci = cpool.tile([T, 1], I32)
            nc.gpsimd.iota(iota_ci[:], pattern=[[1, 1]], base=0, channel_multiplier=1)
            iota_cf = cpool.tile([T, 1], F32)
            nc.vector.tensor_copy(iota_cf[:], iota_ci[:])
            startcol = cpool.tile([T, 1], BF16)
            nc.vector.tensor_scalar(startcol[:], iota_cf[:], float(START), None,
                                    OP.is_equal)
            iota_i = cpool.tile([T, T], I32)
            nc.gpsimd.iota(iota_i[:], pattern=[[1, T]], base=0, channel_multiplier=0)
            iota_f = cpool.tile([T, T], F32)
            nc.vector.tensor_copy(iota_f[:], iota_i[:])
            ident_bf = cpool.tile([T, T], BF16)
            nc.vector.tensor_scalar(ident_bf[:], iota_f[:], iota_cf[:], None,
                                    OP.is_equal)
            ones_bf = cpool.tile([T, 1], BF16)
            nc.vector.memset(ones_bf[:], 1.0)
            ones_f = cpool.tile([T, 1], F32)
            nc.vector.memset(ones_f[:], 1.0)

            # E buffer: [T, k(seg), s(step-in-seg), b]
            E2 = cpool.tile([T, S, SEG, BC], BF16)

            EMACC = cpool.tile([T, BC], F32)
            nc.gpsimd.memset(EMACC[:], 0.0)
            PAIRACC = cpool.tile([T, BC], F32)
            nc.gpsimd.memset(PAIRACC[:], 0.0)
            # masked emission values accumulated over chunks: [p, tl, b, j]
            EGACC = cpool.tile([T, 16, BC, 2], BF16)
            nc.gpsimd.memset(EGACC[:], 0.0)
            egscr = cpool.tile([T, 16, 2], BF16)

            # scan chain inits (log-doubling broadcast of the seed columns)
            aF0 = cpool.tile([T, S, BC], BF16)
            nc.gpsimd.memset(aF0[:], 1.0)
            vB0 = cpool.tile([T, S, BC], BF16)
            nc.gpsimd.memset(vB0[:], 1.0)
            nc.vector.tensor_copy(aF0[:, 0, 0:1], startcol[:])
            nc.vector.tensor_copy(vB0[:, S - 1, 0:1], rcol[:])
            w = 1
            while w < BC:
                nc.vector.tensor_copy(aF0[:, 0, w : 2 * w], aF0[:, 0, 0:w])
                nc.vector.tensor_copy(vB0[:, S - 1, w : 2 * w], vB0[:, S - 1, 0:w])
                w *= 2

            # ---------------- phase 1 + overlapped scan ----------------
            # The scan runs as two independent 16-segment halves. Half 0
            # (segments 0..15, chunks 0..3) is emitted interleaved with the
            # chunk 4..7 staging so it overlaps phase 1 on DVE/PE.
            KPARTS = os.environ.get("KPARTS", "egs")  # e=E-build g=gathers s=scan
            H = S // 2

            xt_pre = {}

            def stage_chunk(c):
                if c in xt_pre:
                    xt = xt_pre.pop(c)
                else:
                    xt = wpool.tile([T, BC, T], BF16, tag="xt", name="xt", bufs=4)
                    nc.sync.dma_start(xt[:], xb_d[c * 128 : (c + 1) * 128])
                gm = wpool.tile([T, 1024], BF16, tag="gm", name="gm")
                nc.sync.dma_start(gm[:], gmask_d[c])
                ei = wpool.tile([T, 32], I16, tag="ei", name="ei")
                nc.sync.dma_start(ei[:], eidx_d[c])

                # transpose 4 sequences at a time into one PSUM bank, then
                # exp straight into E2 (chunk c covers segments 4c..4c+3)
                for q in range(8 if "e" in KPARTS else 0):
                    pst = psT.tile([T, 4 * T], BF16, tag="tp", name="pst")
                    for sbi in range(4):
                        b = 4 * q + sbi
                        nc.tensor.transpose(
                            pst[:, sbi * T : (sbi + 1) * T], xt[:, b, :], ident_bf
                        )
                    # pst free layout (b, t) with t = (k_loc, s)
                    src = pst[:].rearrange("p (b k s) -> p k s b", b=4, k=4, s=SEG)
                    nc.scalar.activation(
                        E2[:, 4 * c : 4 * c + 4, :, 4 * q : 4 * q + 4], src, ACT.Exp
                    )

                # gold emission gather
                if "g" not in KPARTS:
                    return
                eg = wpool.tile([T, 1024], BF16, tag="eg", name="eg")
                nc.gpsimd.ap_gather(eg[:], xt[:], ei[:], channels=T,
                                    num_elems=BC * 64, d=2, num_idxs=512)
                nc.gpsimd.tensor_tensor(eg[:], eg[:], gm[:], OP.mult)
                with nc.allow_low_precision(reason="few-term sums, bf16 ok"):
                    nc.vector.tensor_add(
                        EGACC[:],
                        EGACC[:],
                        eg[:].rearrange("p (t b j) -> p t b j", t=16, b=BC, j=2),
                    )

            # per-half scan state
            aFh = [aF0, None]
            psbh = [None, None]

            # forward chains cover segs 0..30 (f_31 unused); backward chains
            # cover segs 1..31 (g_0 unused) — the edge dummies are trimmed.
            FSL = [slice(0, 16), slice(16, 31)]   # fwd segs per half
            BSL = [slice(1, 16), slice(16, 32)]   # bwd segs per half
            FW = [16, 15]
            BW = [15, 16]

            def scan_slot(h, j):
                # forward: P = W @ aF ; aF' = P * E[:, fseg, j, :]
                psa = psA.tile([T, FW[h], BC], F32, tag=f"A{h}", name="psa")
                src = aF0[:, FSL[h], :] if j == 0 else aFh[h][:]
                nc.tensor.matmul(psa[:], W_sb[:], src)
                aF_new = apool.tile([T, FW[h], BC], BF16, tag=f"aF{h}",
                                    name="aFn")
                nc.vector.tensor_mul(aF_new[:], psa[:], E2[:, FSL[h], j, :])
                aFh[h] = aF_new

                # backward: u = v * E[:, bseg, SEG-1-j, :] ; v' = W2 @ u
                u = apool.tile([T, BW[h], BC], BF16, tag=f"u{h}", name="u")
                vsrc = vB0[:, BSL[h], :] if j == 0 else psbh[h][:]
                nc.vector.tensor_mul(u[:], vsrc, E2[:, BSL[h], SEG - 1 - j, :])
                psb = psB.tile([T, BW[h], BC], F32, tag=f"B{h}", name="psb")
                nc.tensor.matmul(psb[:], W2_sb[:], u[:])
                psbh[h] = psb

            # emission schedule: chunks 0..3 (x DMAs first, back to back so
            # the E build for the scan's first half is fed soonest), then
            # half-0 scan interleaved with chunks 4..7, then half-1 scan,
            # then the pair gather work.
            for c in range(4):
                xt = wpool.tile([T, BC, T], BF16, tag="xt", name="xt",
                                bufs=4)
                nc.sync.dma_start(xt[:], xb_d[c * 128 : (c + 1) * 128])
                xt_pre[c] = xt
            for c in range(4):
                stage_chunk(c)
            nslot = SEG if "s" in KPARTS else 0
            if "e" not in KPARTS:
                nc.vector.memset(E2[:], 0.5)
            for c in range(4, NCH):
                stage_chunk(c)
                for j in range((c - 4) * 6, (c - 3) * 6):
                    if j < nslot:
                        scan_slot(0, j)
            # pair-gather feeds issued only now, behind all x chunks
            nc.sync.dma_start(trep_sb[:], trep_d[:])
            nc.sync.dma_start(pidx_sb[:], pidx_d[:])
            nc.sync.dma_start(pmask_sb[:], pmask_d[:])
            # interleave the remaining half-0 slots with half-1 so the DVE
            # always has two independent chains to alternate between
            rem0 = list(range(24, nslot))
            for j in range(nslot):
                if rem0:
                    scan_slot(0, rem0.pop(0))
                scan_slot(1, j)

            # pair + stop transitions: one gather over the replicated table.
            # k < 4096: (c, tl, b) entries; k >= 4096: len==L stop entries.
            if "g" in KPARTS:
                # emission per-sequence totals via Act accumulators
                for b in range(BC):
                    nc.scalar.activation(
                        egscr[:], EGACC[:, :, b, :], ACT.Copy,
                        accum_out=EMACC[:, b : b + 1],
                    )

                pgb = cpool.tile([T, 129, BC, 2], BF16)
                nc.gpsimd.ap_gather(pgb[:], trep_sb[:], pidx_sb[:], channels=T,
                                    num_elems=8192, d=2, num_idxs=4128)
                nc.gpsimd.tensor_tensor(
                    pgb[:],
                    pgb[:],
                    pmask_sb[:].rearrange("p (c b j) -> p c b j", c=129, j=2),
                    OP.mult,
                )
                # per-sequence totals (pair incl. stop block) via Act accums
                pgscr = cpool.tile([T, 129, 2], BF16)
                for b in range(BC):
                    nc.scalar.activation(
                        pgscr[:], pgb[:, :, b, :], ACT.Copy,
                        accum_out=PAIRACC[:, b : b + 1],
                    )

            # gfin holds g for segs 1..31; aFfin holds f for segs 0..30
            gfin = cpool.tile([T, S - 1, BC], BF16)
            aFfin = cpool.tile([T, S - 1, BC], BF16)
            if psbh[0] is not None:
                nc.scalar.copy(gfin[:, :15, :], psbh[0][:])
                nc.scalar.copy(gfin[:, 15:, :], psbh[1][:])
                nc.scalar.copy(aFfin[:, :16, :], aFh[0][:])
                nc.scalar.copy(aFfin[:, 16:, :], aFh[1][:])
            else:
                nc.vector.memset(gfin[:], 1.0)
                nc.vector.memset(aFfin[:], 1.0)
            aF = aFfin

            # ---------------- junction telescope ----------------
            # d_k = g_{k+1} . f_k (k=0..30), c_k = colsum f_k (k=1..30)
            dm = cpool.tile([T, S - 1, BC], BF16)
            nc.vector.tensor_mul(dm[:], aF[:], gfin[:])
            dr1 = psM.tile([1, 15, BC], F32, tag="m")
            nc.tensor.matmul(dr1[:], ones_bf[:], dm[:, :15, :])
            dr2 = psM.tile([1, 16, BC], F32, tag="m")
            nc.tensor.matmul(dr2[:], ones_bf[:], dm[:, 15:, :])
            lnd = cpool.tile([1, S - 1, BC], F32)
            nc.scalar.activation(lnd[:, :15, :], dr1[:], ACT.Ln)
            nc.scalar.activation(lnd[:, 15:, :], dr2[:], ACT.Ln)

            cr1 = psM.tile([1, 15, BC], F32, tag="m")
            nc.tensor.matmul(cr1[:], ones_bf[:], aF[:, 1:16, :])
            cr2 = psM.tile([1, 15, BC], F32, tag="m")
            nc.tensor.matmul(cr2[:], ones_bf[:], aF[:, 16 : S - 1, :])
            lncs = cpool.tile([1, S - 2, BC], F32)
            nc.scalar.activation(lncs[:, :15, :], cr1[:], ACT.Ln)
            nc.scalar.activation(lncs[:, 15:, :], cr2[:], ACT.Ln)

            dsum = cpool.tile([1, BC], F32)
            nc.vector.tensor_reduce(
                dsum[:], lnd[:].rearrange("p k b -> p b k"), AX.X, OP.add
            )
            csum = cpool.tile([1, BC], F32)
            nc.vector.tensor_reduce(
                csum[:], lncs[:].rearrange("p k b -> p b k"), AX.X, OP.add
            )

            # ---------------- final assembly ----------------
            emr = psM.tile([1, BC], F32, tag="m")
            nc.tensor.matmul(emr[:], ones_f[:], EMACC[:], start=True, stop=False)
            nc.tensor.matmul(emr[:], ones_f[:], PAIRACC[:], start=False, stop=True)

            s2 = cpool.tile([1, BC], F32)
            nc.vector.tensor_sub(s2[:], emr[:], dsum[:])
            res = cpool.tile([1, BC], F32)
            nc.vector.tensor_add(res[:], s2[:], csum[:])
            nc.sync.dma_start(out_d[:].rearrange("(o b) -> o b", o=1), res[:])

    nc.compile()
    return nc


_NC_CACHE = {}


def _get_nc():
    if "nc" not in _NC_CACHE:
        _NC_CACHE["nc"] = build_nc()
    return _NC_CACHE["nc"]


def _host_prep(x, transitions, x_mask, x_len, true_tag):
    """Build all per-core device inputs."""
    B, L, Dd = x.shape
    trans2 = np.asarray(transitions, np.float32).copy()
    trans2[STOP, STOP] = 0.0
    m = np.asarray(x_mask, np.float32)
    mb = m.astype(bool)
    tag = np.asarray(true_tag, np.int64)
    x_len = np.asarray(x_len, np.int64)

    g = float(np.exp(trans2[:D, :D], dtype=np.float64).mean())
    lnik2 = -np.log(126.0 * g) - 0.5  # extra e^{-1/2} ~ E[e^x] for x~N(0,1)

    # padded, masked, normalizer-baked x
    x_pad = np.full((B, L, T), -1e4, np.float32)
    x_pad[:, :, :D] = np.where(mb[:, :, None], x + lnik2, -1e4)
    x_pad[:, :, STOP] = np.where(mb, -1e4, 0.0)
    x_bf = x_pad.astype(BF)

    # gather tables / indices / masks
    transT = np.ascontiguousarray(trans2.T)
    trep = np.broadcast_to(
        transT.reshape(1, T * T).astype(BF), (T, T * T)
    ).copy()

    tagp = np.concatenate(
        [np.full_like(tag[:, :1], START), tag[:, : L - 1]], axis=1
    )
    flat2 = tagp * T + tag  # trans[tag_t, tagp] = transT.flat[flat2]
    # at t == len (< L): the STOP transition trans[STOP, tag_last]
    bi = np.arange(B)
    last = tag[bi, x_len - 1]
    has_slot = x_len < L
    bsel = bi[has_slot]
    flat2[bsel, x_len[bsel]] = last[bsel] * T + STOP
    # pair validity: t < len, plus the stop slot at t == len
    pvalid = np.zeros((B, L), np.float32)
    pvalid[:, :] = m
    pvalid[bsel, x_len[bsel]] = 1.0

    return dict(x_bf=x_bf, trans2=trans2, trep=trep, tag=tag,
                flat2=flat2, pvalid=pvalid, m=m, x_len=x_len)


def _core_inputs(prep, ci):
    s = slice(ci * BC, (ci + 1) * BC)
    tag = prep["tag"][s]
    flat2 = prep["flat2"][s]
    m = prep["m"][s]
    pvalid = prep["pvalid"][s]
    x_len = prep["x_len"][s]

    eidx = np.zeros((NCH, T, 32), np.int16)
    pidx = np.zeros((T, 258), np.int16)

    bb = np.arange(BC)
    row_b = bb % 16          # [32]
    col_g = bb // 16         # [32]
    # emission (per chunk): group g's k = tloc*32 + b lives at
    # [row = 16*g + k%16 (= b%16), col = k//16 (= 2*tloc + b//16)]
    # pair (one shot): group g's k = c*512 + tloc*32 + b lives at
    # [row = 16*g + b%16, col = 32*c + 2*tloc + b//16]; stop k = 4096+b.
    last = tag[bb, x_len - 1]
    for gi in range(8):
        for c in range(NCH):
            for tloc in range(16):
                t = c * 128 + gi * 16 + tloc
                r = gi * 16 + row_b                  # rows [32]
                eidx[c, r, 2 * tloc + col_g] = bb * 64 + (tag[:, t] >> 1)
                pidx[r, 32 * c + 2 * tloc + col_g] = flat2[:, t] >> 1
        pidx[gi * 16 + row_b, 256 + col_g] = np.where(
            x_len == L_FULL, last * 64 + 63, 8191
        )

    # gmask[c, p, tloc*64 + b*2 + j] = (tloc==p%16)*(j==tag&1)*m
    c_i, p_i, tl_i, b_i = np.meshgrid(
        np.arange(NCH), np.arange(T), np.arange(16), bb, indexing="ij")
    t_i = c_i * 128 + (p_i // 16) * 16 + tl_i
    diag = (tl_i == (p_i % 16))
    par = (tag[b_i, t_i] & 1)
    mv = m[b_i, t_i]
    val = (diag * mv).astype(np.float32)  # [NCH, 128, 16, 32]
    gmask = np.zeros((NCH, T, 16, BC, 2), np.float32)
    np.put_along_axis(gmask, par[..., None], val[..., None], axis=4)
    gmask = gmask.reshape(NCH, T, 1024).astype(BF)

    # pmask[p, (c*512 + tloc*32 + b)*2 + j] =
    #   (tloc==p%16)*(j==flat2&1)*pvalid ; stop block: row 0, parity 1
    parp = (flat2[b_i, t_i] & 1)
    pv = pvalid[b_i, t_i]
    pvall = (diag * pv).astype(np.float32)
    pmask5 = np.zeros((NCH, T, 16, BC, 2), np.float32)
    np.put_along_axis(pmask5, parp[..., None], pvall[..., None], axis=4)
    # [c, p, tl, b, j] -> [p, c, tl, b, j]
    pmask = np.zeros((T, 4128, 2), np.float32)
    pmask[:, :4096, :] = pmask5.transpose(1, 0, 2, 3, 4).reshape(T, 4096, 2)
    pmask[0, 4096 + bb, 1] = (x_len == L_FULL).astype(np.float32)
    pmask = pmask.reshape(T, 8256).astype(BF)

    return {
        "x_bf": np.ascontiguousarray(prep["x_bf"][s].transpose(1, 0, 2)),
        "transitions": prep["trans2"],
        "trep": prep["trep"],
        "gmask": gmask,
        "eidx": eidx,
        "pidx": pidx,
        "pmask": pmask,
    }


_LAST_RESULTS = [None]


def kernel(x, transitions, x_mask, x_len, true_tag):
    x = np.asarray(x, np.float32)
    prep = _host_prep(x, transitions, x_mask, x_len, true_tag)
    nc = _get_nc()
    in_maps = [_core_inputs(prep, ci) for ci in range(N_CORES)]
    r = run_bass_kernel_spmd(nc, in_maps, core_ids=list(range(N_CORES)))
    _LAST_RESULTS[0] = r
    return np.concatenate([mm["out"] for mm in r.results]).astype(np.float32)

